# revision 43
# baseline (speedup 1.0000x reference)
"""Masked dot-product attention (B=8, Q=K=2048, D=512) on 8 trn2 NeuronCores.

The reference's masked_softmax replaces logits at masked key positions
(k >= valid_lens[b]) with 0.0 before the softmax, so every masked key
contributes exp(0)=1 * v_k to the numerator and 1 to the denominator.
That tail is a rank-1 term computable in O(K*D) on the host:

    O[b,q] = (sum_{k<L'} e^{s_qk} v_k  +  T'_b) / (sum_{k<L'} e^{s_qk} + C_b)
    T'_b = sum_{k>=L'} v_k,   C_b = K - L',   L' = ceil(L_b/128)*128

(keys in [L, L') get their K^T column zeroed on the host -> score 0 ->
weight exactly 1, with their real v rows, so only k >= L' needs T'/C).

So the device only computes over the first ceil(L_b/128) k-tiles of each
batch -- 68 tiles total here vs 8*16=128 for the dense problem.

Load balancing with ONE uniform SPMD program: the work is organized as
NSLOT identical "slots" per core, each slot = 512 queries x depth_s
k-tiles.  Batches with equal tile counts are PAIRED (batch A on cores
0-3, batch B on cores 4-7, each core takes a 512-query block).  Leftover
batches are SELF-SPLIT along K flash-style: cores 0-3 take the first
half of the k-tiles, cores 4-7 the second half (same query blocks), and
the two partial (numerator, Z) results are summed on the host -- exact,
since no max-subtraction is needed (logits ~ N(0,1), exp is safe fp32).
Odd splits get one zero-padded k-tile (zero K^T -> weight 1, zero V ->
no numerator; the constant 128 it adds to Z is subtracted on the host).

Per slot the device runs:
  phase 1: S^T tiles = K^T-chunk^T @ Q-chunk on TensorE (PSUM f32),
           ScalarE exp -> X^T bf16 in SBUF
  phase 2: num = X^T^T @ V and Z = X^T^T @ ones (shared stationary
           operand), num copied out bf16, Z f32.
Host: gather slots, num_total = sum halves + T', Z_total = sum + C,
O = num/Z, scatter into the full (B, Q, D) f32 output.
"""

import sys

if "/opt/trn_rl_repo" not in sys.path:
    sys.path.insert(0, "/opt/trn_rl_repo")

import numpy as np
import ml_dtypes

BF16 = ml_dtypes.bfloat16

B, SEQ, D = 8, 2048, 512
P = 128
ND = D // P       # 4 contraction chunks
QS = 512          # queries per slot per core
QH = QS // P      # 4 query-halves (psum partitions) per slot
ZN = 8            # ones width for the Z matmul (N=1 exposes LDWEIGHTS)
NCORE = 8
SCALE = 1.0 / float(np.sqrt(D))

_CACHE = {}


# ---------------------------------------------------------------------------
# Scheduling: valid_lens -> uniform slot structure + per-core assignment
# ---------------------------------------------------------------------------

def _schedule(valid_lens):
    """Build the slot schedule.

    Returns (sig, slots) where sig is the hashable compile key (tuple of
    slot depths) and slots is a list of dicts:
      depth: k-tiles per core in this slot
      kind:  'pair' (two batches, full K each) or 'split' (one batch,
             K halved across core groups)
      For 'pair':  ba, bb  (batch for cores 0-3 / 4-7)
      For 'split': b, tiles_a, tiles_b (k-tile ranges), npad_b
    Core c in group g=c//4 handles query block (c%4)*512 of its batch.
    """
    L = [int(x) for x in valid_lens]
    T = [max(1, -(-l // P)) for l in L]  # ceil(L/128), >= 1

    order = sorted(range(B), key=lambda b: -T[b])
    groups = {}
    for b in order:
        groups.setdefault(T[b], []).append(b)

    slots = []
    for depth in sorted(groups, reverse=True):
        bs = groups[depth]
        while len(bs) >= 2:
            ba, bb = bs.pop(0), bs.pop(0)
            slots.append(dict(kind="pair", depth=depth, ba=ba, bb=bb))
        if bs:
            b = bs.pop()
            ta = (depth + 1) // 2
            tb = depth - ta
            slots.append(dict(
                kind="split", depth=ta, b=b,
                tiles_a=(0, ta), tiles_b=(ta, depth), npad_b=ta - tb,
            ))
    slots.sort(key=lambda s: s["depth"])  # ascending: compute starts early
    sig = tuple(s["depth"] for s in slots)
    return sig, slots


# ---------------------------------------------------------------------------
# Bass program (uniform across cores; per-core data differs)
# ---------------------------------------------------------------------------

def _build(sig, repeat=1, phases="pipe"):
    import concourse.bacc as bacc
    import concourse.mybir as mybir
    from concourse.tile import TileContext

    nslot = len(sig)
    ntile = sum(sig)
    toff = [0]
    for t in sig:
        toff.append(toff[-1] + t)
    nc = bacc.Bacc("TRN2")
    qm = nc.dram_tensor("qm", [D, nslot * QS], mybir.dt.bfloat16,
                        kind="ExternalInput")
    ktall = nc.dram_tensor("ktall", [D, ntile * P], mybir.dt.bfloat16,
                           kind="ExternalInput")
    vmall = nc.dram_tensor("vmall", [ntile * P, D], mybir.dt.bfloat16,
                           kind="ExternalInput")
    if phases == "v6":
        # per-slot staging layout: [slot][partition p][h*(D+1) + c],
        # where c==D holds Z (bf16) for query h*128+p of that slot.
        num = nc.dram_tensor("num2", [nslot, P, QH * (D + 1)],
                             mybir.dt.bfloat16, kind="ExternalOutput")
        zden = None
    elif phases == "v7":
        # per-(slot, h) packed rows: [s][h][p][c], c==D holds Z (bf16)
        # for query s*QS + h*P + p.
        num = nc.dram_tensor("num3", [nslot, QH, P, D + 1],
                             mybir.dt.bfloat16, kind="ExternalOutput")
        zden = None
    else:
        num = nc.dram_tensor("num", [nslot * QS, D], mybir.dt.bfloat16,
                             kind="ExternalOutput")
        zden = nc.dram_tensor("zden", [nslot * QS, 1], mybir.dt.float32,
                              kind="ExternalOutput")

    FP32 = mybir.dt.float32
    BF = mybir.dt.bfloat16
    Exp = mybir.ActivationFunctionType.Exp

    if phases == "v2":
        pp_bufs, op_bufs, zp_bufs, out_bufs = 3, 4, 1, 12
    elif phases == "v3":
        pp_bufs, op_bufs, zp_bufs, out_bufs = 2, 4, 2, 12
    elif phases == "v4":
        pp_bufs, op_bufs, zp_bufs, out_bufs = 2, 4, 2, 48
    elif phases in ("v5", "v6"):
        pp_bufs, op_bufs, zp_bufs, out_bufs = 2, 4, 2, 3
    elif phases == "v7":
        pp_bufs, op_bufs, zp_bufs, out_bufs = 4, 2, 1, 12
    else:
        pp_bufs, op_bufs, zp_bufs, out_bufs = 4, 2, 1, 12
    with TileContext(nc) as tc:
        with tc.tile_pool(name="inp", bufs=1) as inp, \
             tc.tile_pool(name="xtp", bufs=1) as xtp, \
             tc.tile_pool(name="pp", bufs=pp_bufs, space="PSUM") as pp, \
             tc.tile_pool(name="op", bufs=op_bufs, space="PSUM") as op, \
             tc.tile_pool(name="zp", bufs=zp_bufs, space="PSUM") as zp, \
             tc.tile_pool(name="outp", bufs=out_bufs) as outp:

            ones = inp.tile([P, ZN], BF, name="ones")
            nc.vector.memset(ones, 1.0)

            # Inputs, issued slot-by-slot so slot 0 compute starts early.
            qts = [inp.tile([P, nslot * QS], BF, name=f"q{d}")
                   for d in range(ND)]
            ktts, vts = [], []
            for s in range(nslot):
                t = sig[s]
                c0 = toff[s] * P
                kt_t = [inp.tile([P, t * P], BF, name=f"kt{s}_{d}")
                        for d in range(ND)]
                v_t = [inp.tile([P, D], BF, name=f"v{s}_{k}")
                       for k in range(t)]
                for d in range(ND):
                    nc.sync.dma_start(
                        qts[d][:, s * QS:(s + 1) * QS],
                        qm[d * P:(d + 1) * P, s * QS:(s + 1) * QS])
                for d in range(ND):
                    nc.sync.dma_start(
                        kt_t[d],
                        ktall[d * P:(d + 1) * P, c0:c0 + t * P])
                for k in range(t):
                    nc.sync.dma_start(
                        v_t[k],
                        vmall[c0 + k * P:c0 + (k + 1) * P, :])
                ktts.append(kt_t)
                vts.append(v_t)

            xts = {}
            if phases in ("p2", "p2noz", "p2nodma"):
                # phase-2-only microbench: x tiles filled once
                for s in range(nslot):
                    for k in range(sig[s]):
                        x = xtp.tile([P, QS], BF, name=f"x{s}_{k}")
                        nc.vector.memset(x, 0.001)
                        xts[(s, k)] = x

            zstage = None
            if phases == "v5":
                zstage = inp.tile([P, nslot * QH], mybir.dt.float32,
                                  name="zstage")

            for _rep in range(repeat):
                if phases == "v7":
                    for s in range(nslot):
                        _phase1(nc, mybir, s, sig[s], qts, ktts[s],
                                xtp, pp, xts)
                        if s > 0:
                            _phase2_v7(nc, mybir, s - 1, sig[s - 1],
                                       vts[s - 1], ones, op, outp,
                                       num, xts)
                    _phase2_v7(nc, mybir, nslot - 1, sig[nslot - 1],
                               vts[nslot - 1], ones, op, outp,
                               num, xts)
                elif phases == "v6":
                    for s in range(nslot):
                        _phase1(nc, mybir, s, sig[s], qts, ktts[s],
                                xtp, pp, xts)
                        if s > 0:
                            _phase2_v6(nc, mybir, s - 1, sig[s - 1],
                                       vts[s - 1], ones, op, zp, outp,
                                       num, xts)
                    _phase2_v6(nc, mybir, nslot - 1, sig[nslot - 1],
                               vts[nslot - 1], ones, op, zp, outp,
                               num, xts)
                elif phases == "v5":
                    for s in range(nslot):
                        _phase1(nc, mybir, s, sig[s], qts, ktts[s],
                                xtp, pp, xts)
                        if s > 0:
                            _phase2_v5(nc, mybir, s - 1, sig[s - 1],
                                       vts[s - 1], ones, op, zp, outp,
                                       num, zstage, xts)
                    _phase2_v5(nc, mybir, nslot - 1, sig[nslot - 1],
                               vts[nslot - 1], ones, op, zp, outp,
                               num, zstage, xts)
                    # one DMA for every slot's Z column:
                    # zden[s*QS + h*P + p] = zstage[p, s*QH + h]
                    nc.sync.dma_start(
                        zden.rearrange("(s h p) c -> p (s h c)",
                                       s=nslot, h=QH, p=P),
                        zstage)
                elif phases in ("v3", "v4"):
                    for s in range(nslot):
                        _phase1(nc, mybir, s, sig[s], qts, ktts[s],
                                xtp, pp, xts)
                        if s > 0:
                            _phase2_v3(nc, mybir, s - 1, sig[s - 1],
                                       vts[s - 1], ones, op, zp, outp,
                                       num, zden, xts)
                    _phase2_v3(nc, mybir, nslot - 1, sig[nslot - 1],
                               vts[nslot - 1], ones, op, zp, outp,
                               num, zden, xts)
                elif phases == "v2":
                    for s in range(nslot):
                        _phase1(nc, mybir, s, sig[s], qts, ktts[s],
                                xtp, pp, xts)
                        if s > 0:
                            _phase2_v2(nc, mybir, s - 1, sig[s - 1],
                                       vts[s - 1], ones, op, zp, outp,
                                       num, zden, xts)
                    _phase2_v2(nc, mybir, nslot - 1, sig[nslot - 1],
                               vts[nslot - 1], ones, op, zp, outp,
                               num, zden, xts)
                elif phases == "pipe":
                    # software pipeline: phase2(s) is emitted after
                    # phase1(s+1) so the trailing exp of slot s is hidden
                    # under the next slot's matmuls.
                    for s in range(nslot):
                        _phase1(nc, mybir, s, sig[s], qts, ktts[s],
                                xtp, pp, xts)
                        if s > 0:
                            _phase2(nc, mybir, s - 1, sig[s - 1], vts[s - 1],
                                    ones, op, outp, num, zden, xts)
                    _phase2(nc, mybir, nslot - 1, sig[nslot - 1],
                            vts[nslot - 1], ones, op, outp, num, zden, xts)
                elif phases == "zsep":
                    for s in range(nslot):
                        _phase1(nc, mybir, s, sig[s], qts, ktts[s],
                                xtp, pp, xts)
                        _phase2_zsep(nc, mybir, s, sig[s], vts[s], ones,
                                     op, outp, num, zden, xts)
                else:
                    for s in range(nslot):
                        if phases in ("serial", "p1"):
                            _phase1(nc, mybir, s, sig[s], qts, ktts[s],
                                    xtp, pp, xts)
                        if phases in ("serial", "p2", "p2noz", "p2nodma"):
                            _phase2(nc, mybir, s, sig[s], vts[s], ones,
                                    op, outp, num, zden, xts,
                                    noz=(phases == "p2noz"),
                                    nodma=(phases == "p2nodma"))

    nc.compile()
    return nc


def _phase1(nc, mybir, s, depth, qts, kt_t, xtp, pp, xts):
    """X^T[k-tile] = exp(scale * K^T-chunk^T Q) for this slot's queries."""
    FP32 = mybir.dt.float32
    BF = mybir.dt.bfloat16
    Exp = mybir.ActivationFunctionType.Exp
    for k in range(depth):
        sp = pp.tile([P, QS], FP32, name="sp")
        for d in range(ND):
            nc.tensor.matmul(
                sp,
                lhsT=kt_t[d][:, k * P:(k + 1) * P],
                rhs=qts[d][:, s * QS:(s + 1) * QS],
                start=(d == 0),
                stop=(d == ND - 1),
            )
        x = xtp.tile([P, QS], BF, name=f"x{s}_{k}")
        nc.scalar.activation(x, sp, Exp, scale=SCALE)
        xts[(s, k)] = x


def _phase2(nc, mybir, s, depth, v_t, ones, op, outp, num, zden, xts,
            noz=False, nodma=False):
    """Per 128-query half: num = X^T.T @ V, Z = X^T.T @ ones."""
    FP32 = mybir.dt.float32
    BF = mybir.dt.bfloat16
    for h in range(QH):
        opsum = op.tile([P, D], FP32, name="opsum")
        zpsum = op.tile([P, ZN], FP32, name="zpsum")
        for k in range(depth):
            w = xts[(s, k)][:, h * P:(h + 1) * P]
            nc.tensor.matmul(opsum, lhsT=w, rhs=v_t[k],
                             start=(k == 0), stop=(k == depth - 1))
            if not noz:
                nc.tensor.matmul(zpsum, lhsT=w, rhs=ones,
                                 start=(k == 0), stop=(k == depth - 1))
        osb = outp.tile([P, D], BF, name="osb")
        nc.vector.tensor_scalar_mul(osb, opsum, 1.0)
        zsb = outp.tile([P, 1], FP32, name="zsb")
        if noz:
            nc.vector.memset(zsb, 1.0)
        else:
            nc.vector.tensor_scalar_add(zsb, zpsum[:, 0:1], 0.0)
        if not nodma:
            row = s * QS + h * P
            nc.sync.dma_start(num[row:row + P, :], osb)
            nc.sync.dma_start(zden[row:row + P, :], zsb)


def _phase2_v7(nc, mybir, s, depth, v_t, ones, op, outp, num3, xts):
    """Like the pipe phase 2 but with Z packed as a 513th bf16 column of
    each h's output tile — one dma_start per h instead of two."""
    FP32 = mybir.dt.float32
    BF = mybir.dt.bfloat16
    for h in range(QH):
        opsum = op.tile([P, D], FP32, name="opsum")
        zpsum = op.tile([P, ZN], FP32, name="zpsum")
        for k in range(depth):
            w = xts[(s, k)][:, h * P:(h + 1) * P]
            nc.tensor.matmul(opsum, lhsT=w, rhs=v_t[k],
                             start=(k == 0), stop=(k == depth - 1))
            nc.tensor.matmul(zpsum, lhsT=w, rhs=ones,
                             start=(k == 0), stop=(k == depth - 1))
        osb = outp.tile([P, D + 1], BF, name="osb")
        nc.vector.tensor_scalar_mul(osb[:, :D], opsum, 1.0)
        nc.vector.tensor_scalar_add(osb[:, D:D + 1], zpsum[:, 0:1], 0.0)
        nc.sync.dma_start(num3[s, h], osb)


def _phase2_v6(nc, mybir, s, depth, v_t, ones, op, zp, outp, num2, xts):
    """Phase 2 with SBUF staging and ONE plain 2D contiguous DMA per
    slot (num + Z packed), minimizing per-dma queue overhead."""
    FP32 = mybir.dt.float32
    BF = mybir.dt.bfloat16
    W = D + 1
    ostage = outp.tile([P, QH * W], BF, name="ostage")
    for h in range(QH):
        opsum = op.tile([P, D], FP32, name="opsum")
        zpsum = zp.tile([P, ZN], FP32, name="zpsum")
        for k in range(depth):
            w = xts[(s, k)][:, h * P:(h + 1) * P]
            nc.tensor.matmul(opsum, lhsT=w, rhs=v_t[k],
                             start=(k == 0), stop=(k == depth - 1))
            nc.tensor.matmul(zpsum, lhsT=w, rhs=ones,
                             start=(k == 0), stop=(k == depth - 1))
        nc.vector.tensor_scalar_mul(ostage[:, h * W:h * W + D], opsum, 1.0)
        nc.vector.tensor_scalar_add(ostage[:, h * W + D:h * W + D + 1],
                                    zpsum[:, 0:1], 0.0)
    nc.sync.dma_start(num2[s], ostage)


def _phase2_v5(nc, mybir, s, depth, v_t, ones, op, zp, outp, num, zstage,
               xts):
    """Phase 2 with outputs staged in SBUF and a single wide num-DMA per
    slot, so the per-h dma_start overhead (~900ns each) disappears."""
    FP32 = mybir.dt.float32
    BF = mybir.dt.bfloat16
    ostage = outp.tile([P, QH * D], BF, name="ostage")
    for h in range(QH):
        opsum = op.tile([P, D], FP32, name="opsum")
        zpsum = zp.tile([P, ZN], FP32, name="zpsum")
        for k in range(depth):
            w = xts[(s, k)][:, h * P:(h + 1) * P]
            nc.tensor.matmul(opsum, lhsT=w, rhs=v_t[k],
                             start=(k == 0), stop=(k == depth - 1))
            nc.tensor.matmul(zpsum, lhsT=w, rhs=ones,
                             start=(k == 0), stop=(k == depth - 1))
        nc.vector.tensor_scalar_mul(ostage[:, h * D:(h + 1) * D], opsum, 1.0)
        nc.vector.tensor_scalar_add(zstage[:, s * QH + h:s * QH + h + 1],
                                    zpsum[:, 0:1], 0.0)
    # num[s*QS + h*P + p, c] = ostage[p, h*D + c]
    nc.sync.dma_start(
        num[s * QS:(s + 1) * QS, :].rearrange("(h p) c -> h p c", h=QH),
        ostage.rearrange("p (h c) -> h p c", h=QH))


def _phase2_v3(nc, mybir, s, depth, v_t, ones, op, zp, outp, num, zden, xts):
    """Phase 2 with 4 rotating pure-opsum banks (a slot's four h-groups
    never reuse a bank, so recycle slack spans a whole phase-1 block)
    and per-h Z tiles from a separate 2-buf pool."""
    FP32 = mybir.dt.float32
    BF = mybir.dt.bfloat16
    for h in range(QH):
        opsum = op.tile([P, D], FP32, name="opsum")
        zpsum = zp.tile([P, ZN], FP32, name="zpsum")
        for k in range(depth):
            w = xts[(s, k)][:, h * P:(h + 1) * P]
            nc.tensor.matmul(opsum, lhsT=w, rhs=v_t[k],
                             start=(k == 0), stop=(k == depth - 1))
            nc.tensor.matmul(zpsum, lhsT=w, rhs=ones,
                             start=(k == 0), stop=(k == depth - 1))
        osb = outp.tile([P, D], BF, name="osb")
        nc.vector.tensor_scalar_mul(osb, opsum, 1.0)
        zsb = outp.tile([P, 1], FP32, name="zsb")
        nc.vector.tensor_scalar_add(zsb, zpsum[:, 0:1], 0.0)
        row = s * QS + h * P
        nc.sync.dma_start(num[row:row + P, :], osb)
        nc.sync.dma_start(zden[row:row + P, :], zsb)


def _phase2_v2(nc, mybir, s, depth, v_t, ones, op, zp, outp, num, zden, xts):
    """Phase 2 with 4 rotating opsum banks and a single per-slot Z psum
    tile (column h = query-half h), so no psum buffer is recycled within
    a slot."""
    FP32 = mybir.dt.float32
    BF = mybir.dt.bfloat16
    zpsum = zp.tile([P, QH * ZN], FP32, name="zpsum")
    for h in range(QH):
        opsum = op.tile([P, D], FP32, name="opsum")
        for k in range(depth):
            w = xts[(s, k)][:, h * P:(h + 1) * P]
            nc.tensor.matmul(opsum, lhsT=w, rhs=v_t[k],
                             start=(k == 0), stop=(k == depth - 1))
            nc.tensor.matmul(zpsum[:, h * ZN:(h + 1) * ZN], lhsT=w,
                             rhs=ones,
                             start=(k == 0), stop=(k == depth - 1))
        osb = outp.tile([P, D], BF, name="osb")
        nc.vector.tensor_scalar_mul(osb, opsum, 1.0)
        row = s * QS + h * P
        nc.sync.dma_start(num[row:row + P, :], osb)
    zsb = outp.tile([P, QH * ZN], FP32, name="zsb")
    nc.vector.tensor_scalar_add(zsb, zpsum, 0.0)
    for h in range(QH):
        row = s * QS + h * P
        nc.sync.dma_start(zden[row:row + P, :], zsb[:, h * ZN:h * ZN + 1])


def _phase2_zsep(nc, mybir, s, depth, v_t, ones, op, outp, num, zden, xts):
    """Phase 2 with the Z matmuls in a separate pass (no per-instruction
    accumulation-group alternation on the opsum path)."""
    FP32 = mybir.dt.float32
    BF = mybir.dt.bfloat16
    for h in range(QH):
        opsum = op.tile([P, D], FP32, name="opsum")
        for k in range(depth):
            w = xts[(s, k)][:, h * P:(h + 1) * P]
            nc.tensor.matmul(opsum, lhsT=w, rhs=v_t[k],
                             start=(k == 0), stop=(k == depth - 1))
        osb = outp.tile([P, D], BF, name="osb")
        nc.vector.tensor_scalar_mul(osb, opsum, 1.0)
        row = s * QS + h * P
        nc.sync.dma_start(num[row:row + P, :], osb)
    for h in range(QH):
        zpsum = op.tile([P, ZN], FP32, name="zpsum")
        for k in range(depth):
            w = xts[(s, k)][:, h * P:(h + 1) * P]
            nc.tensor.matmul(zpsum, lhsT=w, rhs=ones,
                             start=(k == 0), stop=(k == depth - 1))
        zsb = outp.tile([P, 1], FP32, name="zsb")
        nc.vector.tensor_scalar_add(zsb, zpsum[:, 0:1], 0.0)
        row = s * QS + h * P
        nc.sync.dma_start(zden[row:row + P, :], zsb)


def _get_nc(sig, repeat=1, phases="pipe"):
    key = (sig, repeat, phases)
    if key not in _CACHE:
        _CACHE[key] = _build(sig, repeat, phases)
    return _CACHE[key]


# ---------------------------------------------------------------------------
# Host-side data prep / gather
# ---------------------------------------------------------------------------

def _prepare_in_maps(queries, keys, values, valid_lens, slots):
    queries = np.asarray(queries, dtype=np.float32)
    keys = np.asarray(keys, dtype=np.float32)
    values = np.asarray(values, dtype=np.float32)
    L = [int(x) for x in np.asarray(valid_lens).reshape(-1)]
    assert queries.shape == (B, SEQ, D)

    # Per-batch masked K^T (f32, columns >= L zeroed), transposed Q.
    ktb = []
    qtb = []
    for b in range(B):
        kt = np.ascontiguousarray(keys[b].T)
        if L[b] < SEQ:
            kt[:, L[b]:] = 0.0
        ktb.append(kt)
        qtb.append(np.ascontiguousarray(queries[b].T).astype(BF16))

    ntile = sum(sl["depth"] for sl in slots)
    in_maps = []
    for c in range(NCORE):
        g, qb = c // 4, c % 4
        q_parts = []
        ktm = np.zeros((D, ntile * P), dtype=np.float32)
        vm = np.zeros((ntile * P, D), dtype=np.float32)
        col = 0
        for s, sl in enumerate(slots):
            t = sl["depth"]
            if sl["kind"] == "pair":
                b = sl["ba"] if g == 0 else sl["bb"]
                k0, k1 = 0, t
            else:
                b = sl["b"]
                k0, k1 = sl["tiles_a"] if g == 0 else sl["tiles_b"]
            q_parts.append(qtb[b][:, qb * QS:(qb + 1) * QS])
            nk = (k1 - k0) * P
            ktm[:, col:col + nk] = ktb[b][:, k0 * P:k1 * P]
            vm[col:col + nk] = values[b][k0 * P:k1 * P]
            col += t * P
        m = {
            "qm": np.concatenate(q_parts, axis=1),
            "ktall": ktm.astype(BF16),
            "vmall": vm.astype(BF16),
        }
        in_maps.append(m)
    return in_maps


def _unpack_core(o, nslot):
    """Return (num [nslot*QS, D], zden [nslot*QS, 1]) f32 from a per-core
    output dict (either the packed 'num2' layout or 'num'/'zden')."""
    if "num3" in o:
        a = np.asarray(o["num3"]).astype(np.float32)  # [nslot, QH, P, D+1]
        a = a.reshape(nslot * QS, D + 1)
        return a[:, :D], a[:, D:]
    if "num2" in o:
        a = np.asarray(o["num2"]).astype(np.float32)  # [nslot, P, QH*(D+1)]
        a = a.reshape(nslot, P, QH, D + 1).transpose(0, 2, 1, 3)
        a = a.reshape(nslot * QS, D + 1)
        return a[:, :D], a[:, D:]
    return (np.asarray(o["num"]).astype(np.float32),
            np.asarray(o["zden"]).astype(np.float32))


def _gather(outs, slots, values, valid_lens):
    """outs: per-core output dicts (see _unpack_core)."""
    values = np.asarray(values, dtype=np.float32)
    L = [int(x) for x in np.asarray(valid_lens).reshape(-1)]
    nslot = len(slots)
    unpacked = [_unpack_core(o, nslot) for o in outs]
    O = np.empty((B, SEQ, D), dtype=np.float32)
    for s, sl in enumerate(slots):
        t = sl["depth"]
        r0, r1 = s * QS, (s + 1) * QS
        if sl["kind"] == "pair":
            for g, b in ((0, sl["ba"]), (1, sl["bb"])):
                Lp = t * P
                Tp = values[b][Lp:].sum(axis=0)
                C = SEQ - Lp
                for qb in range(4):
                    c = g * 4 + qb
                    n = unpacked[c][0][r0:r1]
                    z = unpacked[c][1][r0:r1]
                    O[b, qb * QS:(qb + 1) * QS] = (n + Tp) / (z + C)
        else:
            b = sl["b"]
            k0a, k1a = sl["tiles_a"]
            k0b, k1b = sl["tiles_b"]
            Lp = k1b * P  # end of real tiles
            Tp = values[b][Lp:].sum(axis=0)
            C = (SEQ - Lp) - P * sl["npad_b"]
            for qb in range(4):
                ca, cb = qb, 4 + qb
                na, za = (u[r0:r1] for u in unpacked[ca])
                nb, zb = (u[r0:r1] for u in unpacked[cb])
                O[b, qb * QS:(qb + 1) * QS] = (na + nb + Tp) / (za + zb + C)
    return O


# ---------------------------------------------------------------------------
# Entry point
# ---------------------------------------------------------------------------

PHASES = "v7"


def _run(queries, keys, values, valid_lens, trace=False):
    from concourse import bass_utils

    sig, slots = _schedule(valid_lens)
    nc = _get_nc(sig, phases=PHASES)
    in_maps = _prepare_in_maps(queries, keys, values, valid_lens, slots)
    res = bass_utils.run_bass_kernel_spmd(
        nc, in_maps, core_ids=list(range(NCORE)), trace=trace
    )
    out = _gather(res.results, slots, values, valid_lens)
    return out, res


def kernel(queries, keys, values, valid_lens):
    out, _ = _run(queries, keys, values, valid_lens, trace=False)
    return out
